# revision 68
# baseline (speedup 1.0000x reference)
"""Trainium2 Bass kernel for nn_AugmentedAttentionHead.

See kernel design notes: data-parallel over batch (8/core); transposed
softmax ([k-part, q-free]) so exp() output feeds attn@v directly as the
stationary operand; log-space Gaussian bias reconstructed by one selector
matmul per k-chunk; x host-pre-transposed; f32r matmuls; ln/exp-only ACT
table; prefix kills folded into host tables.

The per-batch work is split into 5 phases and traced along anti-diagonals
(software pipelining across batches) so each engine always has another
batch's independent work to fill dependency stalls.
"""

import numpy as np
from contextlib import ExitStack

B, T, E, H = 64, 577, 768, 64
GRID = 24
EPS = 1e-5
NCORES = 8
BPC = B // NCORES

TSZ = [128, 128, 128, 128, 65]
TOFF = [0, 128, 256, 384, 512]
NCH = 5
FW = 50
NEG = -1.0e30
NPH = 5


def _host_consts(w_q, w_k, w_v, w_sigma, w_alpha):
    import ml_dtypes

    bf16 = ml_dtypes.bfloat16
    wqkv = np.concatenate([w_q, w_k, w_v], axis=1)
    w_ext = np.ascontiguousarray(
        wqkv.reshape(6, 128, 192).transpose(1, 0, 2).reshape(128, 6 * 192)
    ).astype(bf16)

    w_sa = np.zeros((64, 4), bf16)
    w_sa[:, 0:2] = (8.0 * w_sigma).astype(bf16)
    # negated so the sigma exp (scale=-1) and alpha exp share one activation
    w_sa[:, 2] = (-8.0 * w_alpha[:, 0]).astype(bf16)

    dxy2s = np.zeros((128, NCH * FW), np.float32)
    for c in range(NCH):
        for r in range(TSZ[c]):
            t = TOFF[c] + r
            o = c * FW
            dxy2s[r, o + 49] = NEG
            if t == 0:
                dxy2s[r, o:o + 24] = NEG
                dxy2s[r, o + 25:o + 49] = NEG
                continue
            p = t - 1
            py, px = p // GRID, p % GRID
            j = np.arange(GRID, dtype=np.float32)
            dxy2s[r, o:o + 24] = -0.5 * (py - j) ** 2
            dxy2s[r, o + 25:o + 49] = -0.5 * (px - j) ** 2

    sel = np.zeros((FW, NCH * 128), bf16)
    for c in range(NCH):
        for r in range(TSZ[c]):
            t = TOFF[c] + r
            if t == 0:
                sel[49, c * 128 + r] = 1.0
                continue
            p = t - 1
            sel[p // GRID, c * 128 + r] = 1.0
            sel[24, c * 128 + r] = 1.0
            sel[25 + p % GRID, c * 128 + r] = 1.0

    ident = np.eye(128, dtype=np.float32)
    ident_bf = np.eye(128, dtype=bf16)
    onz = np.stack([np.ones(128), np.zeros(128)], 1).astype(bf16)
    return w_ext, w_sa, dxy2s, sel, ident, ident_bf, onz


def _trace(nc, tc, ctx, consts_f, need_gb):
    import concourse.mybir as mybir

    dt = mybir.dt
    AF = mybir.ActivationFunctionType
    OP = mybir.AluOpType
    bs0, bs1, ba0 = consts_f[:3]

    xT_d = nc.dram_tensor("xT", [BPC, 6, 128, T], dt.bfloat16, kind="ExternalInput").ap()
    wext_d = nc.dram_tensor("w_ext", [128, 6 * 192], dt.bfloat16, kind="ExternalInput").ap()
    wsa_d = nc.dram_tensor("w_sa", [64, 4], dt.bfloat16, kind="ExternalInput").ap()
    dxy_d = nc.dram_tensor("dxy2s", [128, NCH * FW], dt.float32, kind="ExternalInput").ap()
    sel_d = nc.dram_tensor("sel", [FW, NCH * 128], dt.bfloat16, kind="ExternalInput").ap()
    id_d = nc.dram_tensor("ident", [128, 128], dt.float32r, kind="ExternalInput").ap()
    idbf_d = nc.dram_tensor("ident_bf", [128, 128], dt.bfloat16, kind="ExternalInput").ap()
    onz_d = nc.dram_tensor("onz", [128, 2], dt.bfloat16, kind="ExternalInput").ap()
    if need_gb:
        gb_d = nc.dram_tensor("gb", [4, 64], dt.float32, kind="ExternalInput").ap()
    out_d = nc.dram_tensor("out", [BPC, T, H], dt.float32, kind="ExternalOutput").ap()

    cpool = ctx.enter_context(tc.tile_pool(name="consts", bufs=1))
    xpool = ctx.enter_context(tc.tile_pool(name="x", bufs=4))
    wkpool = ctx.enter_context(tc.tile_pool(name="work", bufs=8))
    rpool = ctx.enter_context(tc.tile_pool(name="raw", bufs=15))
    qkpool = ctx.enter_context(tc.tile_pool(name="qk", bufs=10))
    vpool = ctx.enter_context(tc.tile_pool(name="v", bufs=25))
    apool = ctx.enter_context(tc.tile_pool(name="attn", bufs=15))
    spool = ctx.enter_context(tc.tile_pool(name="small", bufs=4))
    opool = ctx.enter_context(tc.tile_pool(name="outb", bufs=3))

    # PSUM (8 banks): {qkv, transpose, sigma-alpha, attn@v} x2 (1-bank slots)
    # + bias/sim x3 (2-bank slots)
    ps_sm = ctx.enter_context(tc.tile_pool(name="ps_sm", bufs=2, space="PSUM"))
    ps_zl = ctx.enter_context(tc.tile_pool(name="ps_zl", bufs=3, space="PSUM"))

    # w_ext first (gates batch-0 QKV); remaining consts are DMA'd after the
    # first batch's x so the pipeline can start ~5us earlier.
    w_ext = cpool.tile([128, 6 * 192], dt.bfloat16)
    nc.sync.dma_start(w_ext[:], wext_d)

    def late_consts_a():
        # needed by phase2(0), which runs in slot 1: these MUST be emitted
        # before slot 1 so Tile sees the writer before the first reader
        # (a reader traced before its writer gets no dependency = race)
        nc.sync.dma_start(onz[:], onz_d)
        nc.sync.dma_start(ident_bf[:], idbf_d)
        nc.sync.dma_start(w_sa[:], wsa_d)
        if need_gb:
            nc.sync.dma_start(gb[:], gb_d)

    def late_consts_b():
        # needed by phase3(0)/phase4(0) (slots 2/3): emitted end of slot 1
        nc.sync.dma_start(dxy2s[:], dxy_d)
        nc.sync.dma_start(sel[:], sel_d)

    w_sa = cpool.tile([64, 4], dt.bfloat16)
    dxy2s = cpool.tile([128, NCH * FW], dt.float32)
    sel = cpool.tile([FW, NCH * 128], dt.bfloat16)
    ident_bf = cpool.tile([128, 128], dt.bfloat16)
    onz = cpool.tile([128, 2], dt.bfloat16)
    gb = None
    if need_gb:
        gb = cpool.tile([4, 64], dt.float32)

    _bias_cache = {}

    def fbias(val, tsz=128):
        val = float(val)
        if val == 0.0:
            return 0.0
        if val not in _bias_cache:
            bt = cpool.tile([128, 1], dt.float32, name=f"bias{len(_bias_cache)}")
            nc.vector.memset(bt[:], val)
            _bias_cache[val] = bt
        return _bias_cache[val][0:tsz, :]

    def phase1(S, b):
        """x DMA, QKV matmuls, raw evac, LN stats."""
        S["xt"] = xt = xpool.tile([128, 6 * T], dt.bfloat16, tag="xT",
                                  name=f"x{b}")
        if b == 0:
            for h in range(2):
                nc.sync.dma_start(
                    xt[:, h * 3 * T:(h + 1) * 3 * T]
                    .rearrange("p (j t) -> p j t", j=3),
                    xT_d[b, 3 * h:3 * h + 3].rearrange("j p t -> p j t"))
        else:
            nc.sync.dma_start(
                xt[:].rearrange("p (j t) -> p j t", j=6),
                xT_d[b].rearrange("j p t -> p j t"))
        S["raw"] = raw = [
            rpool.tile([128, 192], dt.float32, tag="raw", name=f"raw{b}_{i}")
            for i in range(NCH)]
        S["mv"] = mv = spool.tile([128, 20], dt.float32, tag="mv", name=f"mv{b}")
        for c in range(NCH):
            tsz, toff = TSZ[c], TOFF[c]
            pqkv = ps_sm.tile([128, 256], dt.float32, tag="sm", name=f"qkv{b}_{c}")
            for j in range(6):
                nc.tensor.matmul(
                    pqkv[0:tsz, 0:192], xt[:, j * T + toff:j * T + toff + tsz],
                    w_ext[:, j * 192:(j + 1) * 192],
                    start=(j == 0), stop=(j == 5),
                )
            nc.vector.tensor_copy(raw[c][0:tsz, :], pqkv[0:tsz, 0:192])
            st = spool.tile([128, 12], dt.float32, tag="st", name=f"st{b}_{c}")
            nc.vector.bn_stats(st[0:tsz, 0:6], raw[c][0:tsz, 0:64])
            nc.vector.bn_stats(st[0:tsz, 6:12], raw[c][0:tsz, 64:128])
            nc.vector.bn_aggr(mv[0:tsz, 4 * c:4 * c + 2], st[0:tsz, 0:6])
            nc.vector.bn_aggr(mv[0:tsz, 4 * c + 2:4 * c + 4], st[0:tsz, 6:12])

    def phase2(S, b):
        """LN params + apply, v build, q/k transposes into qkT."""
        mv = S["mv"]
        raw = S["raw"]
        mv4 = mv[:].rearrange("p (c f) -> p c f", f=4)
        lnv = spool.tile([128, 10], dt.float32, tag="lnv", name=f"lnv{b}")
        lnv2 = lnv[:].rearrange("p (c f) -> p c f", f=2)
        sc = spool.tile([128, 10], dt.float32, tag="sc", name=f"sc{b}")
        sc2 = sc[:].rearrange("p (c f) -> p c f", f=2)
        nmr = spool.tile([128, 10], dt.float32, tag="nmr", name=f"nmr{b}")
        nmr2 = nmr[:].rearrange("p (c f) -> p c f", f=2)
        # scale the q-variance by 64 so exp(-0.5*ln(64*var)) = 0.125*rsqrt:
        # folds the 1/sqrt(H) softmax scale without a separate biased exp.
        # batch 0 streams in two chunk groups to start its applies earlier.
        for lo, hi in ([(0, 3), (3, 5)] if b == 0 else [(0, NCH)]):
            nc.gpsimd.tensor_scalar(mv4[:, lo:hi, 1:2], mv4[:, lo:hi, 1:2],
                                    64.0, None, OP.mult)
            nc.scalar.activation(lnv2[:, lo:hi, :], mv4[:, lo:hi, 1::2],
                                 AF.Ln, bias=fbias(EPS))
            nc.scalar.activation(sc2[:, lo:hi, :], lnv2[:, lo:hi, :], AF.Exp,
                                 scale=-0.5)
            nc.gpsimd.scalar_tensor_tensor(
                nmr2[:, lo:hi, :], mv4[:, lo:hi, 0::2], -1.0, sc2[:, lo:hi, :],
                OP.mult, OP.mult)

        S["qkT"] = qkT = wkpool.tile([64, 1156], dt.bfloat16, tag="qkT",
                                     name=f"qkT{b}")
        S["v_ext"] = v_ext = [
            vpool.tile([128, 66], dt.bfloat16, tag="vext", name=f"vext{b}_{i}")
            for i in range(NCH)]
        for c in range(NCH):
            tsz = TSZ[c]
            t2 = tsz + (tsz & 1)
            qk_ln = qkpool.tile([128, 128], dt.bfloat16, tag="qkln",
                                name=f"qkln{b}_{c}")
            if t2 != tsz:
                nc.vector.tensor_copy(
                    qk_ln[64:66, :], onz[64:66, 1:2].broadcast_to([2, 128]))
            nc.vector.tensor_scalar(
                qk_ln[0:tsz, 0:64], raw[c][0:tsz, 0:64],
                sc[0:tsz, 2 * c:2 * c + 1], nmr[0:tsz, 2 * c:2 * c + 1],
                OP.mult, OP.add)
            nc.vector.tensor_scalar(
                qk_ln[0:tsz, 64:128], raw[c][0:tsz, 64:128],
                sc[0:tsz, 2 * c + 1:2 * c + 2], nmr[0:tsz, 2 * c + 1:2 * c + 2],
                OP.mult, OP.add)
            if need_gb:
                nc.vector.tensor_mul(qk_ln[0:tsz, 0:64], qk_ln[0:tsz, 0:64],
                                     gb[0:1, :].partition_broadcast(tsz))
                nc.vector.tensor_add(qk_ln[0:tsz, 0:64], qk_ln[0:tsz, 0:64],
                                     gb[1:2, :].partition_broadcast(tsz))
                nc.vector.tensor_mul(qk_ln[0:tsz, 64:128], qk_ln[0:tsz, 64:128],
                                     gb[2:3, :].partition_broadcast(tsz))
                nc.vector.tensor_add(qk_ln[0:tsz, 64:128], qk_ln[0:tsz, 64:128],
                                     gb[3:4, :].partition_broadcast(tsz))
            nc.gpsimd.tensor_scalar(
                v_ext[c][0:tsz, 0:64], raw[c][0:tsz, 128:192], 1.0, None, OP.mult)
            nc.gpsimd.tensor_copy(v_ext[c][0:tsz, 64:66], onz[0:tsz, :])
            S[f"qkln{c}"] = qk_ln

        # transposes + evac by chunk pair: one PSUM tile and one copy per
        # pair halves the DVE evac instruction count
        for c0, cw in ((0, 2), (2, 2), (4, 1)):
            toff = TOFF[c0]
            tpc = ps_sm.tile([128, 512], dt.bfloat16, tag="sm", name=f"tp{b}_{c0}")
            ncols = 0
            for i in range(cw):
                qk_ln = S.pop(f"qkln{c0 + i}")
                t2 = TSZ[c0 + i] + (TSZ[c0 + i] & 1)
                nc.tensor.transpose(tpc[0:64, 128 * i:128 * i + t2],
                                    qk_ln[0:t2, 0:64], ident_bf[0:t2, 0:t2])
                nc.tensor.transpose(tpc[0:64, 256 + 128 * i:256 + 128 * i + t2],
                                    qk_ln[0:t2, 64:128], ident_bf[0:t2, 0:t2])
                ncols += min(128, 578 - TOFF[c0 + i])
            dst = qkT[:].rearrange("p (g q) -> p g q", g=2)[:, :, toff:toff + ncols]
            nc.vector.tensor_copy(
                dst,
                tpc[0:64, :].rearrange("p (g q) -> p g q", g=2)[:, :, 0:ncols])

        # sigma/alpha MLP runs here (a slot earlier than its phase3
        # consumers) so the serial ACT chain texp/spe/spl is never on the
        # critical path of phase3's factor build + transposes
        psa = ps_sm.tile([128, 256], dt.float32, tag="sm", name=f"psa{b}")
        for c in range(NCH):
            nc.tensor.matmul(
                psa[0:TSZ[c], 4 * c:4 * c + 4], qkT[:, TOFF[c]:TOFF[c] + TSZ[c]],
                w_sa[:], start=True, stop=True)
        sap = spool.tile([128, 20], dt.float32, tag="sap", name=f"sap{b}")
        nc.vector.tensor_copy(sap[:], psa[:, 0:20])
        sap4 = sap[:].rearrange("p (c f) -> p c f", f=4)
        texp = S["texp"] = spool.tile([128, 15], dt.float32, tag="texp",
                                      name=f"texp{b}")
        texp3 = texp[:].rearrange("p (c f) -> p c f", f=3)
        if bs0 == bs1 == -ba0:
            # one exp covers sigma cols (e^{-x-b}) and alpha col (e^{x+b},
            # via the host-side negated w_alpha)
            nc.scalar.activation(texp3[:, :, :], sap4[:, :, 0:3], AF.Exp,
                                 bias=fbias(-bs0), scale=-1.0)
        else:
            for col in range(2):
                nc.scalar.activation(texp3[:, :, col], sap4[:, :, col], AF.Exp,
                                     bias=fbias(-(bs0 if col == 0 else bs1)),
                                     scale=-1.0)
            nc.scalar.activation(texp3[:, :, 2], sap4[:, :, 2], AF.Exp,
                                 bias=fbias(ba0), scale=-1.0)
        S["spl"] = spl = spool.tile([128, 5], dt.float32, tag="spl",
                                    name=f"spl{b}")
        nc.scalar.activation(spl[:], texp3[:, :, 2], AF.Ln, bias=fbias(1.0))

    def phase3(S, b):
        """log-space factor build, factor transposes."""
        texp, spl = S["texp"], S["spl"]
        ab = spool.tile([128, 10], dt.float32, tag="ab", name=f"ab{b}")
        ab2 = ab[:].rearrange("p (c f) -> p c f", f=2)
        texp3 = S["texp"][:].rearrange("p (c f) -> p c f", f=3)
        nc.vector.tensor_scalar_add(ab2[:, :, :], texp3[:, :, 0:2], 1.0)
        nc.vector.tensor_mul(ab[:], ab[:], ab[:])

        yn = spool.tile([128, NCH * FW], dt.bfloat16, tag="yn", name=f"yn{b}")
        dxy3 = dxy2s[:].rearrange("p (c f) -> p c f", f=FW)
        yn3 = yn[:].rearrange("p (c f) -> p c f", f=FW)
        ab3 = ab[:].rearrange("p (c f) -> p c f", f=2)
        nc.gpsimd.scalar_tensor_tensor(
            yn3[:, :, 0:24], dxy3[:, :, 0:24], 1.0,
            ab3[:, :, 1:2].broadcast_to([128, NCH, 24]), OP.mult, OP.mult)
        nc.gpsimd.scalar_tensor_tensor(
            yn3[:, :, 25:50], dxy3[:, :, 25:50], 1.0,
            ab3[:, :, 0:1].broadcast_to([128, NCH, 25]), OP.mult, OP.mult)
        # lna written straight into yn col 24 by ACT (saves a DVE copy that
        # head-of-line blocked the vector queue on the ACT chain)
        nc.scalar.activation(yn3[:, :, 24:25], spl[:].unsqueeze(-1),
                             AF.Ln, scale=0.125)

        S["yT"] = yT = wkpool.tile([FW, 578], dt.bfloat16, tag="yT", name=f"yT{b}")
        for c0, cw in ((0, 2), (2, 2), (4, 1)):
            pfc = ps_sm.tile([128, 256], dt.bfloat16, tag="sm", name=f"pf{b}_{c0}")
            ncols = 0
            for i in range(cw):
                c = c0 + i
                t2 = TSZ[c] + (TSZ[c] & 1)
                nc.tensor.transpose(pfc[0:FW, 128 * i:128 * i + t2],
                                    yn[0:t2, c * FW:(c + 1) * FW],
                                    ident_bf[0:t2, 0:t2])
                ncols += min(128, 578 - TOFF[c])
            nc.vector.tensor_copy(yT[:, TOFF[c0]:TOFF[c0] + ncols],
                                  pfc[0:FW, 0:ncols])

    def phase4_range(S, b, c_lo, c_hi):
        """main loop: sim^T + bias + exp per k-chunk (transposed softmax).

        pl/pz are 2-bank PSUM tiles: cols [0:320] in bank A hold q 0:319,
        cols [512:770] in bank B hold q 320:577, so one strided activation
        [p, 2, 320] covers both matmul outputs (SBUF cols land q-aligned;
        tail cols >=578 are junk and never read).
        """
        qkT, yT = S["qkT"], S["yT"]
        if c_lo == 0:
            S["attnT"] = [
                apool.tile([128, 578], dt.bfloat16, tag="attnT",
                           name=f"attnT{b}_{i}")
                for i in range(NCH)]
        attnT = S["attnT"]

        # software pipeline: chunk c's bias-add + softmax-exp are emitted one
        # iteration late, so the in-order PE/ACT queues never stall waiting on
        # each other (PE's ident-add needs ACT's exp(pl); ACT's exp(pz) needs
        # PE's ident-add). The last chunk's flush is deferred to the end of
        # the schedule slot (after phase5 of the older batch) for the same
        # reason.
        pend = S.pop("pend4", None)

        def flush(pend):
            c, tsz, expl, pz = pend
            nc.tensor.matmul(pz[0:tsz, 0:289], ident_bf[0:tsz, 0:tsz],
                             expl[0:tsz, 0:289], start=False, stop=True)
            nc.tensor.matmul(pz[0:tsz, 512:801], ident_bf[0:tsz, 0:tsz],
                             expl[0:tsz, 289:578], start=False, stop=True)
            pz3 = pz[:].rearrange("p (s f) -> p s f", f=512)
            at3 = attnT[c][:].rearrange("p (s f) -> p s f", f=289)
            nc.scalar.activation(at3[0:tsz, :, :], pz3[0:tsz, :, 0:289], AF.Exp)

        S["flush4"] = flush
        for c in range(c_lo, c_hi):
            tsz, toff = TSZ[c], TOFF[c]
            selc = sel[:, c * 128:c * 128 + tsz]
            kTc = qkT[:, 578 + toff:578 + toff + tsz]
            expl = wkpool.tile([128, 578], dt.bfloat16, tag="expl",
                               name=f"expl{b}_{c}")
            pl = ps_zl.tile([128, 1024], dt.float32, tag="zl", name=f"pl{b}{c}")
            nc.tensor.matmul(pl[0:tsz, 0:289], selc, yT[:, 0:289],
                             start=True, stop=True)
            nc.tensor.matmul(pl[0:tsz, 512:801], selc, yT[:, 289:578],
                             start=True, stop=True)
            pl3 = pl[:].rearrange("p (s f) -> p s f", f=512)
            ex3 = expl[:].rearrange("p (s f) -> p s f", f=289)
            nc.scalar.activation(ex3[0:tsz, :, :], pl3[0:tsz, :, 0:289], AF.Exp)
            pz = ps_zl.tile([128, 1024], dt.float32, tag="zl", name=f"pz{b}{c}")
            nc.tensor.matmul(pz[0:tsz, 0:289], kTc, qkT[:, 0:289],
                             start=True, stop=False)
            nc.tensor.matmul(pz[0:tsz, 512:801], kTc, qkT[:, 289:578],
                             start=True, stop=False)
            if pend is not None:
                flush(pend)
            pend = (c, tsz, expl, pz)
        S["pend4"] = pend

    def phase5(S, b):
        """attn @ [v|1] + batched normalize + store."""
        if "pend4" in S:
            S["flush4"](S.pop("pend4"))
        attnT, v_ext = S["attnT"], S["v_ext"]
        osb = opool.tile([128, 320], dt.float32, tag="osb", name=f"osb{b}")
        for g, qcs in enumerate(((0, 1), (2, 3), (4,))):
            po = ps_sm.tile([128, 256], dt.float32, tag="sm", name=f"po{b}_{g}")
            for i, qc in enumerate(qcs):
                qsz, qoff = TSZ[qc], TOFF[qc]
                for kc in range(NCH):
                    nc.tensor.matmul(
                        po[0:qsz, 128 * i:128 * i + 66],
                        attnT[kc][0:TSZ[kc], qoff:qoff + qsz],
                        v_ext[kc][0:TSZ[kc], :], start=(kc == 0), stop=(kc == 4))
            n = len(qcs)
            po5 = po[:].rearrange("p (c f) -> p c f", f=128)[:, 0:n, :]
            rcp = spool.tile([128, 2], dt.float32, tag="rcp", name=f"rcp{b}_{g}")
            nc.vector.reciprocal(rcp[:, 0:n], po5[:, :, 64])
            osb3 = osb[:].rearrange("p (c f) -> p c f", f=64)[:, 2 * g:2 * g + n, :]
            nc.vector.scalar_tensor_tensor(
                osb3, po5[:, :, 0:64], 1.0,
                rcp[:, 0:n].unsqueeze(-1).broadcast_to([128, n, 64]),
                OP.mult, OP.mult)
            # store per group so the tail DMA overlaps the next group's
            # matmuls instead of waiting for the whole batch
            if g < 2:
                nc.sync.dma_start(
                    out_d[b, 256 * g:256 * g + 256, :]
                    .rearrange("(c p) h -> p c h", p=128),
                    osb[:, 128 * g:128 * g + 128]
                    .rearrange("p (c h) -> p c h", h=64))
            else:
                nc.sync.dma_start(out_d[b, 512:T, :], osb[0:65, 256:320])

    # anti-diagonal software pipeline across batches
    # slot-internal emission order sets in-order queue priorities; found by
    # sweep (phase5 and phase3 early keeps DVE/ACT queues from head-of-line
    # blocking behind the newest batch's work). Batch 0 runs on compressed
    # lags (phase3 shares phase2's slot) since its serial chain IS the
    # pipeline-fill critical path; phase4's deferred tail flush happens at
    # the head of phase5.
    import os
    ORDER = [int(ch) for ch in os.environ.get("SLOT_ORDER", "15342")]
    stages = {1: phase1, 2: phase2, 3: phase3,
              4: lambda S, b: phase4_range(S, b, 0, NCH), 5: phase5}
    LAG = {1: 0, 2: 1, 3: 2, 4: 3, 5: 4}
    LAG0 = {1: 0, 2: 1, 3: 2, 4: 3, 5: 4}

    def slot_of(b, p):
        return LAG0[p] if b == 0 else b + LAG[p]

    states = [dict() for _ in range(BPC)]
    for slot in range(BPC + NPH - 1):
        for p in ORDER:
            for b in range(BPC):
                if slot_of(b, p) != slot:
                    continue
                stages[p](states[b], b)
        if slot == 0:
            late_consts_a()
        elif slot == 1:
            late_consts_b()


_CACHE = {}


def _patch_act_tables():
    # bacc's insert_act_table_loads maps each activation func to the first
    # table containing it, which makes Exp<->Ln transitions reload tables
    # (1.28 us each, ~30x per kernel). Restrict the funcs this kernel uses
    # to the combined natural_log_exp_and_others set so one load suffices.
    import concourse.bacc as bacc_mod
    import concourse.mybir as mybir
    from concourse.hw_specs import get_activation_tables as _gat
    if getattr(bacc_mod, "_ant_act_tables_patched", False):
        return
    AF = mybir.ActivationFunctionType
    mine = {AF.Exp, AF.Ln, AF.Copy, AF.Identity, AF.MemsetZero}

    def patched(arch):
        tabs = _gat(arch)
        combined = tabs.get("natural_log_exp_and_others")
        if combined and mine <= combined:
            for name, s in tabs.items():
                if name != "natural_log_exp_and_others":
                    tabs[name] = s - mine
        return tabs

    bacc_mod.get_activation_tables = patched
    bacc_mod._ant_act_tables_patched = True


def _build(consts_f, need_gb):
    import concourse.tile as tile
    from concourse import bacc

    _patch_act_tables()
    key = (consts_f, need_gb)
    if key in _CACHE:
        return _CACHE[key]
    nc = bacc.Bacc("TRN2", target_bir_lowering=False, debug=False)
    with tile.TileContext(nc) as tc, ExitStack() as ctx:
        _trace(nc, tc, ctx, consts_f, need_gb)
    nc.finalize()
    _CACHE[key] = nc
    return nc


def kernel(x, w_q, w_k, w_v, q_gamma, q_beta, k_gamma, k_beta,
           w_sigma, b_sigma, w_alpha, b_alpha):
    from concourse import bass_utils

    x = np.asarray(x, np.float32)
    w_q, w_k, w_v = (np.asarray(a, np.float32) for a in (w_q, w_k, w_v))
    w_sigma = np.asarray(w_sigma, np.float32)
    w_alpha = np.asarray(w_alpha, np.float32)
    b_sigma = np.asarray(b_sigma, np.float32)
    b_alpha = np.asarray(b_alpha, np.float32)
    q_gamma, q_beta = np.asarray(q_gamma, np.float32), np.asarray(q_beta, np.float32)
    k_gamma, k_beta = np.asarray(k_gamma, np.float32), np.asarray(k_beta, np.float32)

    trivial_gb = (
        np.allclose(q_gamma, 1) and np.allclose(k_gamma, 1)
        and np.allclose(q_beta, 0) and np.allclose(k_beta, 0)
    )

    w_ext, w_sa, dxy2s, sel, ident, ident_bf, onz = _host_consts(
        w_q, w_k, w_v, w_sigma, w_alpha)
    consts_f = (float(b_sigma[0]), float(b_sigma[1]), float(b_alpha[0]))
    nc = _build(consts_f, not trivial_gb)

    import ml_dtypes

    xt = np.ascontiguousarray(
        x.reshape(NCORES, BPC, T, E).transpose(0, 1, 3, 2)
    ).astype(ml_dtypes.bfloat16).reshape(NCORES, BPC, 6, 128, T)

    base = {
        "w_ext": w_ext, "w_sa": w_sa, "dxy2s": dxy2s, "sel": sel, "ident": ident,
        "ident_bf": ident_bf, "onz": onz,
    }
    if not trivial_gb:
        base["gb"] = np.stack(
            [q_gamma, q_beta / 8.0, k_gamma, k_beta]).astype(np.float32)
    in_maps = [{**base, "xT": xt[c]} for c in range(NCORES)]

    res = bass_utils.run_bass_kernel_spmd(nc, in_maps, core_ids=list(range(NCORES)))
    out = np.concatenate([res.results[c]["out"] for c in range(NCORES)], axis=0)
    return out.astype(np.float32)



# revision 69
# speedup vs baseline: 1.0091x; 1.0091x over previous
"""Trainium2 Bass kernel for nn_AugmentedAttentionHead.

See kernel design notes: data-parallel over batch (8/core); transposed
softmax ([k-part, q-free]) so exp() output feeds attn@v directly as the
stationary operand; log-space Gaussian bias reconstructed by one selector
matmul per k-chunk; x host-pre-transposed; f32r matmuls; ln/exp-only ACT
table; prefix kills folded into host tables.

The per-batch work is split into 5 phases and traced along anti-diagonals
(software pipelining across batches) so each engine always has another
batch's independent work to fill dependency stalls.
"""

import numpy as np
from contextlib import ExitStack

B, T, E, H = 64, 577, 768, 64
GRID = 24
EPS = 1e-5
NCORES = 8
BPC = B // NCORES

TSZ = [128, 128, 128, 128, 65]
TOFF = [0, 128, 256, 384, 512]
NCH = 5
FW = 50
NEG = -1.0e30
NPH = 5


def _host_consts(w_q, w_k, w_v, w_sigma, w_alpha):
    import ml_dtypes

    bf16 = ml_dtypes.bfloat16
    wqkv = np.concatenate([w_q, w_k, w_v], axis=1)
    w_ext = np.ascontiguousarray(
        wqkv.reshape(6, 128, 192).transpose(1, 0, 2).reshape(128, 6 * 192)
    ).astype(bf16)

    w_sa = np.zeros((64, 4), bf16)
    w_sa[:, 0:2] = (8.0 * w_sigma).astype(bf16)
    # negated so the sigma exp (scale=-1) and alpha exp share one activation
    w_sa[:, 2] = (-8.0 * w_alpha[:, 0]).astype(bf16)

    dxy2s = np.zeros((128, NCH * FW), np.float32)
    for c in range(NCH):
        for r in range(TSZ[c]):
            t = TOFF[c] + r
            o = c * FW
            dxy2s[r, o + 49] = NEG
            if t == 0:
                dxy2s[r, o:o + 24] = NEG
                dxy2s[r, o + 25:o + 49] = NEG
                continue
            p = t - 1
            py, px = p // GRID, p % GRID
            j = np.arange(GRID, dtype=np.float32)
            dxy2s[r, o:o + 24] = -0.5 * (py - j) ** 2
            dxy2s[r, o + 25:o + 49] = -0.5 * (px - j) ** 2

    sel = np.zeros((FW, NCH * 128), bf16)
    for c in range(NCH):
        for r in range(TSZ[c]):
            t = TOFF[c] + r
            if t == 0:
                sel[49, c * 128 + r] = 1.0
                continue
            p = t - 1
            sel[p // GRID, c * 128 + r] = 1.0
            sel[24, c * 128 + r] = 1.0
            sel[25 + p % GRID, c * 128 + r] = 1.0

    ident = np.eye(128, dtype=np.float32)
    ident_bf = np.eye(128, dtype=bf16)
    onz = np.stack([np.ones(128), np.zeros(128)], 1).astype(bf16)
    return w_ext, w_sa, dxy2s, sel, ident, ident_bf, onz


def _trace(nc, tc, ctx, consts_f, need_gb):
    import concourse.mybir as mybir

    dt = mybir.dt
    AF = mybir.ActivationFunctionType
    OP = mybir.AluOpType
    bs0, bs1, ba0 = consts_f[:3]

    xT_d = nc.dram_tensor("xT", [BPC, 6, 128, T], dt.bfloat16, kind="ExternalInput").ap()
    wext_d = nc.dram_tensor("w_ext", [128, 6 * 192], dt.bfloat16, kind="ExternalInput").ap()
    wsa_d = nc.dram_tensor("w_sa", [64, 4], dt.bfloat16, kind="ExternalInput").ap()
    dxy_d = nc.dram_tensor("dxy2s", [128, NCH * FW], dt.float32, kind="ExternalInput").ap()
    sel_d = nc.dram_tensor("sel", [FW, NCH * 128], dt.bfloat16, kind="ExternalInput").ap()
    id_d = nc.dram_tensor("ident", [128, 128], dt.float32r, kind="ExternalInput").ap()
    idbf_d = nc.dram_tensor("ident_bf", [128, 128], dt.bfloat16, kind="ExternalInput").ap()
    onz_d = nc.dram_tensor("onz", [128, 2], dt.bfloat16, kind="ExternalInput").ap()
    if need_gb:
        gb_d = nc.dram_tensor("gb", [4, 64], dt.float32, kind="ExternalInput").ap()
    out_d = nc.dram_tensor("out", [BPC, T, H], dt.float32, kind="ExternalOutput").ap()

    cpool = ctx.enter_context(tc.tile_pool(name="consts", bufs=1))
    xpool = ctx.enter_context(tc.tile_pool(name="x", bufs=4))
    wkpool = ctx.enter_context(tc.tile_pool(name="work", bufs=8))
    rpool = ctx.enter_context(tc.tile_pool(name="raw", bufs=15))
    qkpool = ctx.enter_context(tc.tile_pool(name="qk", bufs=10))
    vpool = ctx.enter_context(tc.tile_pool(name="v", bufs=25))
    apool = ctx.enter_context(tc.tile_pool(name="attn", bufs=15))
    spool = ctx.enter_context(tc.tile_pool(name="small", bufs=4))
    opool = ctx.enter_context(tc.tile_pool(name="outb", bufs=3))

    # PSUM (8 banks): {qkv, transpose, sigma-alpha, attn@v} x2 (1-bank slots)
    # + bias/sim x3 (2-bank slots)
    ps_sm = ctx.enter_context(tc.tile_pool(name="ps_sm", bufs=2, space="PSUM"))
    ps_zl = ctx.enter_context(tc.tile_pool(name="ps_zl", bufs=3, space="PSUM"))

    # w_ext first (gates batch-0 QKV); remaining consts are DMA'd after the
    # first batch's x so the pipeline can start ~5us earlier.
    w_ext = cpool.tile([128, 6 * 192], dt.bfloat16)
    nc.sync.dma_start(w_ext[:], wext_d)

    def late_consts_a():
        # needed by phase2(0), which runs in slot 1: these MUST be emitted
        # before slot 1 so Tile sees the writer before the first reader
        # (a reader traced before its writer gets no dependency = race)
        nc.sync.dma_start(onz[:], onz_d)
        nc.sync.dma_start(ident_bf[:], idbf_d)
        nc.sync.dma_start(w_sa[:], wsa_d)
        if need_gb:
            nc.sync.dma_start(gb[:], gb_d)

    def late_consts_b():
        # needed by phase3(0)/phase4(0) (slots 2/3): emitted end of slot 1
        nc.sync.dma_start(dxy2s[:], dxy_d)
        nc.sync.dma_start(sel[:], sel_d)

    w_sa = cpool.tile([64, 4], dt.bfloat16)
    dxy2s = cpool.tile([128, NCH * FW], dt.float32)
    sel = cpool.tile([FW, NCH * 128], dt.bfloat16)
    ident_bf = cpool.tile([128, 128], dt.bfloat16)
    onz = cpool.tile([128, 2], dt.bfloat16)
    gb = None
    if need_gb:
        gb = cpool.tile([4, 64], dt.float32)

    _bias_cache = {}

    def fbias(val, tsz=128):
        val = float(val)
        if val == 0.0:
            return 0.0
        if val not in _bias_cache:
            bt = cpool.tile([128, 1], dt.float32, name=f"bias{len(_bias_cache)}")
            nc.vector.memset(bt[:], val)
            _bias_cache[val] = bt
        return _bias_cache[val][0:tsz, :]

    def phase1(S, b):
        """x DMA, QKV matmuls, raw evac, LN stats."""
        S["xt"] = xt = xpool.tile([128, 6 * T], dt.bfloat16, tag="xT",
                                  name=f"x{b}")
        if b == 0:
            for h in range(2):
                nc.sync.dma_start(
                    xt[:, h * 3 * T:(h + 1) * 3 * T]
                    .rearrange("p (j t) -> p j t", j=3),
                    xT_d[b, 3 * h:3 * h + 3].rearrange("j p t -> p j t"))
        else:
            nc.sync.dma_start(
                xt[:].rearrange("p (j t) -> p j t", j=6),
                xT_d[b].rearrange("j p t -> p j t"))
        S["raw"] = raw = [
            rpool.tile([128, 192], dt.float32, tag="raw", name=f"raw{b}_{i}")
            for i in range(NCH)]
        S["mv"] = mv = spool.tile([128, 20], dt.float32, tag="mv", name=f"mv{b}")
        for c in range(NCH):
            tsz, toff = TSZ[c], TOFF[c]
            pqkv = ps_sm.tile([128, 256], dt.float32, tag="sm", name=f"qkv{b}_{c}")
            for j in range(6):
                nc.tensor.matmul(
                    pqkv[0:tsz, 0:192], xt[:, j * T + toff:j * T + toff + tsz],
                    w_ext[:, j * 192:(j + 1) * 192],
                    start=(j == 0), stop=(j == 5),
                )
            nc.vector.tensor_copy(raw[c][0:tsz, :], pqkv[0:tsz, 0:192])
            st = spool.tile([128, 12], dt.float32, tag="st", name=f"st{b}_{c}")
            nc.vector.bn_stats(st[0:tsz, 0:6], raw[c][0:tsz, 0:64])
            nc.vector.bn_stats(st[0:tsz, 6:12], raw[c][0:tsz, 64:128])
            nc.vector.bn_aggr(mv[0:tsz, 4 * c:4 * c + 2], st[0:tsz, 0:6])
            nc.vector.bn_aggr(mv[0:tsz, 4 * c + 2:4 * c + 4], st[0:tsz, 6:12])

    def phase2(S, b):
        """LN params + apply, v build, q/k transposes into qkT."""
        mv = S["mv"]
        raw = S["raw"]
        mv4 = mv[:].rearrange("p (c f) -> p c f", f=4)
        lnv = spool.tile([128, 10], dt.float32, tag="lnv", name=f"lnv{b}")
        lnv2 = lnv[:].rearrange("p (c f) -> p c f", f=2)
        sc = spool.tile([128, 10], dt.float32, tag="sc", name=f"sc{b}")
        sc2 = sc[:].rearrange("p (c f) -> p c f", f=2)
        nmr = spool.tile([128, 10], dt.float32, tag="nmr", name=f"nmr{b}")
        nmr2 = nmr[:].rearrange("p (c f) -> p c f", f=2)
        # scale the q-variance by 64 so exp(-0.5*ln(64*var)) = 0.125*rsqrt:
        # folds the 1/sqrt(H) softmax scale without a separate biased exp.
        # batch 0 streams in two chunk groups to start its applies earlier.
        for lo, hi in ([(0, 3), (3, 5)] if b == 0 else [(0, NCH)]):
            nc.gpsimd.tensor_scalar(mv4[:, lo:hi, 1:2], mv4[:, lo:hi, 1:2],
                                    64.0, None, OP.mult)
            nc.scalar.activation(lnv2[:, lo:hi, :], mv4[:, lo:hi, 1::2],
                                 AF.Ln, bias=fbias(EPS))
            nc.scalar.activation(sc2[:, lo:hi, :], lnv2[:, lo:hi, :], AF.Exp,
                                 scale=-0.5)
            nc.gpsimd.scalar_tensor_tensor(
                nmr2[:, lo:hi, :], mv4[:, lo:hi, 0::2], -1.0, sc2[:, lo:hi, :],
                OP.mult, OP.mult)

        S["qkT"] = qkT = wkpool.tile([64, 1156], dt.bfloat16, tag="qkT",
                                     name=f"qkT{b}")
        S["v_ext"] = v_ext = [
            vpool.tile([128, 66], dt.bfloat16, tag="vext", name=f"vext{b}_{i}")
            for i in range(NCH)]
        for c in range(NCH):
            tsz = TSZ[c]
            t2 = tsz + (tsz & 1)
            qk_ln = qkpool.tile([128, 128], dt.bfloat16, tag="qkln",
                                name=f"qkln{b}_{c}")
            if t2 != tsz:
                nc.vector.tensor_copy(
                    qk_ln[64:66, :], onz[64:66, 1:2].broadcast_to([2, 128]))
            nc.vector.tensor_scalar(
                qk_ln[0:tsz, 0:64], raw[c][0:tsz, 0:64],
                sc[0:tsz, 2 * c:2 * c + 1], nmr[0:tsz, 2 * c:2 * c + 1],
                OP.mult, OP.add)
            nc.vector.tensor_scalar(
                qk_ln[0:tsz, 64:128], raw[c][0:tsz, 64:128],
                sc[0:tsz, 2 * c + 1:2 * c + 2], nmr[0:tsz, 2 * c + 1:2 * c + 2],
                OP.mult, OP.add)
            if need_gb:
                nc.vector.tensor_mul(qk_ln[0:tsz, 0:64], qk_ln[0:tsz, 0:64],
                                     gb[0:1, :].partition_broadcast(tsz))
                nc.vector.tensor_add(qk_ln[0:tsz, 0:64], qk_ln[0:tsz, 0:64],
                                     gb[1:2, :].partition_broadcast(tsz))
                nc.vector.tensor_mul(qk_ln[0:tsz, 64:128], qk_ln[0:tsz, 64:128],
                                     gb[2:3, :].partition_broadcast(tsz))
                nc.vector.tensor_add(qk_ln[0:tsz, 64:128], qk_ln[0:tsz, 64:128],
                                     gb[3:4, :].partition_broadcast(tsz))
            nc.gpsimd.tensor_scalar(
                v_ext[c][0:tsz, 0:64], raw[c][0:tsz, 128:192], 1.0, None, OP.mult)
            nc.gpsimd.tensor_copy(v_ext[c][0:tsz, 64:66], onz[0:tsz, :])
            S[f"qkln{c}"] = qk_ln

        # transposes + evac by chunk pair: one PSUM tile and one copy per
        # pair halves the DVE evac instruction count
        for c0, cw in ((0, 2), (2, 2), (4, 1)):
            toff = TOFF[c0]
            tpc = ps_sm.tile([128, 512], dt.bfloat16, tag="sm", name=f"tp{b}_{c0}")
            ncols = 0
            for i in range(cw):
                qk_ln = S.pop(f"qkln{c0 + i}")
                t2 = TSZ[c0 + i] + (TSZ[c0 + i] & 1)
                nc.tensor.transpose(tpc[0:64, 128 * i:128 * i + t2],
                                    qk_ln[0:t2, 0:64], ident_bf[0:t2, 0:t2])
                nc.tensor.transpose(tpc[0:64, 256 + 128 * i:256 + 128 * i + t2],
                                    qk_ln[0:t2, 64:128], ident_bf[0:t2, 0:t2])
                ncols += min(128, 578 - TOFF[c0 + i])
            dst = qkT[:].rearrange("p (g q) -> p g q", g=2)[:, :, toff:toff + ncols]
            nc.vector.tensor_copy(
                dst,
                tpc[0:64, :].rearrange("p (g q) -> p g q", g=2)[:, :, 0:ncols])

        # sigma/alpha MLP runs here (a slot earlier than its phase3
        # consumers) so the serial ACT chain texp/spe/spl is never on the
        # critical path of phase3's factor build + transposes
        psa = ps_sm.tile([128, 256], dt.float32, tag="sm", name=f"psa{b}")
        for c in range(NCH):
            nc.tensor.matmul(
                psa[0:TSZ[c], 4 * c:4 * c + 4], qkT[:, TOFF[c]:TOFF[c] + TSZ[c]],
                w_sa[:], start=True, stop=True)
        sap = spool.tile([128, 20], dt.float32, tag="sap", name=f"sap{b}")
        nc.vector.tensor_copy(sap[:], psa[:, 0:20])
        sap4 = sap[:].rearrange("p (c f) -> p c f", f=4)
        texp = S["texp"] = spool.tile([128, 15], dt.float32, tag="texp",
                                      name=f"texp{b}")
        texp3 = texp[:].rearrange("p (c f) -> p c f", f=3)
        if bs0 == bs1 == -ba0:
            # one exp covers sigma cols (e^{-x-b}) and alpha col (e^{x+b},
            # via the host-side negated w_alpha)
            nc.scalar.activation(texp3[:, :, :], sap4[:, :, 0:3], AF.Exp,
                                 bias=fbias(-bs0), scale=-1.0)
        else:
            for col in range(2):
                nc.scalar.activation(texp3[:, :, col], sap4[:, :, col], AF.Exp,
                                     bias=fbias(-(bs0 if col == 0 else bs1)),
                                     scale=-1.0)
            nc.scalar.activation(texp3[:, :, 2], sap4[:, :, 2], AF.Exp,
                                 bias=fbias(ba0), scale=-1.0)
        S["spl"] = spl = spool.tile([128, 5], dt.float32, tag="spl",
                                    name=f"spl{b}")
        nc.scalar.activation(spl[:], texp3[:, :, 2], AF.Ln, bias=fbias(1.0))

    def phase3(S, b):
        """log-space factor build, factor transposes."""
        texp, spl = S["texp"], S["spl"]
        ab = spool.tile([128, 10], dt.float32, tag="ab", name=f"ab{b}")
        ab2 = ab[:].rearrange("p (c f) -> p c f", f=2)
        texp3 = S["texp"][:].rearrange("p (c f) -> p c f", f=3)
        nc.vector.tensor_scalar_add(ab2[:, :, :], texp3[:, :, 0:2], 1.0)
        nc.vector.tensor_mul(ab[:], ab[:], ab[:])

        yn = spool.tile([128, NCH * FW], dt.bfloat16, tag="yn", name=f"yn{b}")
        dxy3 = dxy2s[:].rearrange("p (c f) -> p c f", f=FW)
        yn3 = yn[:].rearrange("p (c f) -> p c f", f=FW)
        ab3 = ab[:].rearrange("p (c f) -> p c f", f=2)
        nc.gpsimd.scalar_tensor_tensor(
            yn3[:, :, 0:24], dxy3[:, :, 0:24], 1.0,
            ab3[:, :, 1:2].broadcast_to([128, NCH, 24]), OP.mult, OP.mult)
        nc.vector.scalar_tensor_tensor(
            yn3[:, :, 25:50], dxy3[:, :, 25:50], 1.0,
            ab3[:, :, 0:1].broadcast_to([128, NCH, 25]), OP.mult, OP.mult)
        # lna written straight into yn col 24 by ACT (saves a DVE copy that
        # head-of-line blocked the vector queue on the ACT chain)
        nc.scalar.activation(yn3[:, :, 24:25], spl[:].unsqueeze(-1),
                             AF.Ln, scale=0.125)

        S["yT"] = yT = wkpool.tile([FW, 578], dt.bfloat16, tag="yT", name=f"yT{b}")
        for c0, cw in ((0, 2), (2, 2), (4, 1)):
            pfc = ps_sm.tile([128, 256], dt.bfloat16, tag="sm", name=f"pf{b}_{c0}")
            ncols = 0
            for i in range(cw):
                c = c0 + i
                t2 = TSZ[c] + (TSZ[c] & 1)
                nc.tensor.transpose(pfc[0:FW, 128 * i:128 * i + t2],
                                    yn[0:t2, c * FW:(c + 1) * FW],
                                    ident_bf[0:t2, 0:t2])
                ncols += min(128, 578 - TOFF[c])
            nc.vector.tensor_copy(yT[:, TOFF[c0]:TOFF[c0] + ncols],
                                  pfc[0:FW, 0:ncols])

    def phase4_range(S, b, c_lo, c_hi):
        """main loop: sim^T + bias + exp per k-chunk (transposed softmax).

        pl/pz are 2-bank PSUM tiles: cols [0:320] in bank A hold q 0:319,
        cols [512:770] in bank B hold q 320:577, so one strided activation
        [p, 2, 320] covers both matmul outputs (SBUF cols land q-aligned;
        tail cols >=578 are junk and never read).
        """
        qkT, yT = S["qkT"], S["yT"]
        if c_lo == 0:
            S["attnT"] = [
                apool.tile([128, 578], dt.bfloat16, tag="attnT",
                           name=f"attnT{b}_{i}")
                for i in range(NCH)]
        attnT = S["attnT"]

        # software pipeline: chunk c's bias-add + softmax-exp are emitted one
        # iteration late, so the in-order PE/ACT queues never stall waiting on
        # each other (PE's ident-add needs ACT's exp(pl); ACT's exp(pz) needs
        # PE's ident-add). The last chunk's flush is deferred to the end of
        # the schedule slot (after phase5 of the older batch) for the same
        # reason.
        pend = S.pop("pend4", None)

        def flush(pend):
            c, tsz, expl, pz = pend
            nc.tensor.matmul(pz[0:tsz, 0:289], ident_bf[0:tsz, 0:tsz],
                             expl[0:tsz, 0:289], start=False, stop=True)
            nc.tensor.matmul(pz[0:tsz, 512:801], ident_bf[0:tsz, 0:tsz],
                             expl[0:tsz, 289:578], start=False, stop=True)
            pz3 = pz[:].rearrange("p (s f) -> p s f", f=512)
            at3 = attnT[c][:].rearrange("p (s f) -> p s f", f=289)
            nc.scalar.activation(at3[0:tsz, :, :], pz3[0:tsz, :, 0:289], AF.Exp)

        S["flush4"] = flush
        for c in range(c_lo, c_hi):
            tsz, toff = TSZ[c], TOFF[c]
            selc = sel[:, c * 128:c * 128 + tsz]
            kTc = qkT[:, 578 + toff:578 + toff + tsz]
            expl = wkpool.tile([128, 578], dt.bfloat16, tag="expl",
                               name=f"expl{b}_{c}")
            pl = ps_zl.tile([128, 1024], dt.float32, tag="zl", name=f"pl{b}{c}")
            nc.tensor.matmul(pl[0:tsz, 0:289], selc, yT[:, 0:289],
                             start=True, stop=True)
            nc.tensor.matmul(pl[0:tsz, 512:801], selc, yT[:, 289:578],
                             start=True, stop=True)
            pl3 = pl[:].rearrange("p (s f) -> p s f", f=512)
            ex3 = expl[:].rearrange("p (s f) -> p s f", f=289)
            nc.scalar.activation(ex3[0:tsz, :, :], pl3[0:tsz, :, 0:289], AF.Exp)
            pz = ps_zl.tile([128, 1024], dt.float32, tag="zl", name=f"pz{b}{c}")
            nc.tensor.matmul(pz[0:tsz, 0:289], kTc, qkT[:, 0:289],
                             start=True, stop=False)
            nc.tensor.matmul(pz[0:tsz, 512:801], kTc, qkT[:, 289:578],
                             start=True, stop=False)
            if pend is not None:
                flush(pend)
            pend = (c, tsz, expl, pz)
        S["pend4"] = pend

    def phase5(S, b):
        """attn @ [v|1] + batched normalize + store."""
        if "pend4" in S:
            S["flush4"](S.pop("pend4"))
        attnT, v_ext = S["attnT"], S["v_ext"]
        osb = opool.tile([128, 320], dt.float32, tag="osb", name=f"osb{b}")
        for g, qcs in enumerate(((0, 1), (2, 3), (4,))):
            po = ps_sm.tile([128, 256], dt.float32, tag="sm", name=f"po{b}_{g}")
            for i, qc in enumerate(qcs):
                qsz, qoff = TSZ[qc], TOFF[qc]
                for kc in range(NCH):
                    nc.tensor.matmul(
                        po[0:qsz, 128 * i:128 * i + 66],
                        attnT[kc][0:TSZ[kc], qoff:qoff + qsz],
                        v_ext[kc][0:TSZ[kc], :], start=(kc == 0), stop=(kc == 4))
            n = len(qcs)
            po5 = po[:].rearrange("p (c f) -> p c f", f=128)[:, 0:n, :]
            rcp = spool.tile([128, 2], dt.float32, tag="rcp", name=f"rcp{b}_{g}")
            nc.vector.reciprocal(rcp[:, 0:n], po5[:, :, 64])
            osb3 = osb[:].rearrange("p (c f) -> p c f", f=64)[:, 2 * g:2 * g + n, :]
            nc.vector.scalar_tensor_tensor(
                osb3, po5[:, :, 0:64], 1.0,
                rcp[:, 0:n].unsqueeze(-1).broadcast_to([128, n, 64]),
                OP.mult, OP.mult)
            # store per group so the tail DMA overlaps the next group's
            # matmuls instead of waiting for the whole batch
            if g < 2:
                nc.sync.dma_start(
                    out_d[b, 256 * g:256 * g + 256, :]
                    .rearrange("(c p) h -> p c h", p=128),
                    osb[:, 128 * g:128 * g + 128]
                    .rearrange("p (c h) -> p c h", h=64))
            else:
                nc.sync.dma_start(out_d[b, 512:T, :], osb[0:65, 256:320])

    # anti-diagonal software pipeline across batches
    # slot-internal emission order sets in-order queue priorities; found by
    # sweep (phase5 and phase3 early keeps DVE/ACT queues from head-of-line
    # blocking behind the newest batch's work). Batch 0 runs on compressed
    # lags (phase3 shares phase2's slot) since its serial chain IS the
    # pipeline-fill critical path; phase4's deferred tail flush happens at
    # the head of phase5.
    import os
    ORDER = [int(ch) for ch in os.environ.get("SLOT_ORDER", "15342")]
    stages = {1: phase1, 2: phase2, 3: phase3,
              4: lambda S, b: phase4_range(S, b, 0, NCH), 5: phase5}
    LAG = {1: 0, 2: 1, 3: 2, 4: 3, 5: 4}
    LAG0 = {1: 0, 2: 1, 3: 2, 4: 3, 5: 4}

    def slot_of(b, p):
        return LAG0[p] if b == 0 else b + LAG[p]

    states = [dict() for _ in range(BPC)]
    for slot in range(BPC + NPH - 1):
        for p in ORDER:
            for b in range(BPC):
                if slot_of(b, p) != slot:
                    continue
                stages[p](states[b], b)
        if slot == 0:
            late_consts_a()
        elif slot == 1:
            late_consts_b()


_CACHE = {}


def _patch_act_tables():
    # bacc's insert_act_table_loads maps each activation func to the first
    # table containing it, which makes Exp<->Ln transitions reload tables
    # (1.28 us each, ~30x per kernel). Restrict the funcs this kernel uses
    # to the combined natural_log_exp_and_others set so one load suffices.
    import concourse.bacc as bacc_mod
    import concourse.mybir as mybir
    from concourse.hw_specs import get_activation_tables as _gat
    if getattr(bacc_mod, "_ant_act_tables_patched", False):
        return
    AF = mybir.ActivationFunctionType
    mine = {AF.Exp, AF.Ln, AF.Copy, AF.Identity, AF.MemsetZero}

    def patched(arch):
        tabs = _gat(arch)
        combined = tabs.get("natural_log_exp_and_others")
        if combined and mine <= combined:
            for name, s in tabs.items():
                if name != "natural_log_exp_and_others":
                    tabs[name] = s - mine
        return tabs

    bacc_mod.get_activation_tables = patched
    bacc_mod._ant_act_tables_patched = True


def _build(consts_f, need_gb):
    import concourse.tile as tile
    from concourse import bacc

    _patch_act_tables()
    key = (consts_f, need_gb)
    if key in _CACHE:
        return _CACHE[key]
    nc = bacc.Bacc("TRN2", target_bir_lowering=False, debug=False)
    with tile.TileContext(nc) as tc, ExitStack() as ctx:
        _trace(nc, tc, ctx, consts_f, need_gb)
    nc.finalize()
    _CACHE[key] = nc
    return nc


def kernel(x, w_q, w_k, w_v, q_gamma, q_beta, k_gamma, k_beta,
           w_sigma, b_sigma, w_alpha, b_alpha):
    from concourse import bass_utils

    x = np.asarray(x, np.float32)
    w_q, w_k, w_v = (np.asarray(a, np.float32) for a in (w_q, w_k, w_v))
    w_sigma = np.asarray(w_sigma, np.float32)
    w_alpha = np.asarray(w_alpha, np.float32)
    b_sigma = np.asarray(b_sigma, np.float32)
    b_alpha = np.asarray(b_alpha, np.float32)
    q_gamma, q_beta = np.asarray(q_gamma, np.float32), np.asarray(q_beta, np.float32)
    k_gamma, k_beta = np.asarray(k_gamma, np.float32), np.asarray(k_beta, np.float32)

    trivial_gb = (
        np.allclose(q_gamma, 1) and np.allclose(k_gamma, 1)
        and np.allclose(q_beta, 0) and np.allclose(k_beta, 0)
    )

    w_ext, w_sa, dxy2s, sel, ident, ident_bf, onz = _host_consts(
        w_q, w_k, w_v, w_sigma, w_alpha)
    consts_f = (float(b_sigma[0]), float(b_sigma[1]), float(b_alpha[0]))
    nc = _build(consts_f, not trivial_gb)

    import ml_dtypes

    xt = np.ascontiguousarray(
        x.reshape(NCORES, BPC, T, E).transpose(0, 1, 3, 2)
    ).astype(ml_dtypes.bfloat16).reshape(NCORES, BPC, 6, 128, T)

    base = {
        "w_ext": w_ext, "w_sa": w_sa, "dxy2s": dxy2s, "sel": sel, "ident": ident,
        "ident_bf": ident_bf, "onz": onz,
    }
    if not trivial_gb:
        base["gb"] = np.stack(
            [q_gamma, q_beta / 8.0, k_gamma, k_beta]).astype(np.float32)
    in_maps = [{**base, "xT": xt[c]} for c in range(NCORES)]

    res = bass_utils.run_bass_kernel_spmd(nc, in_maps, core_ids=list(range(NCORES)))
    out = np.concatenate([res.results[c]["out"] for c in range(NCORES)], axis=0)
    return out.astype(np.float32)

